# revision 2
# baseline (speedup 1.0000x reference)
"""Fused cross-attention audio fuser (dense transformer block) on TRN2.

Strategy: pure batch data-parallelism across the 8 NeuronCores (B=16 -> 2
batches per core, no collectives). Per batch everything is kept
channel-major ([C, tokens]) so the only transposes needed are 8 PE block
transposes of the audio features:

  qT = Wq.T @ imgT (+bq, *scale)      [C_AUD, HW]     (imgT is native layout)
  kT = Wk.T @ audT (+bk)              [C_AUD, K_LEN]
  v  = audT.T @ Wv (+bv via K=1 mm)   [K_LEN, C_AUD]  (seq-major)
  S_hT = kT_h.T @ qT_h                [K_LEN, HW] per head (K=64, row-packed pairs)
  expS = exp(S_hT)                    (no max subtraction; scores are provably small)
  sumexp_hT = ones.T @ expS           (matmul reduction+partition broadcast in one)
  attnT_h = v_h.T @ expS              (col-tiled head pairs -> [128, HW] chunks)
  attnT normalized by DVE reciprocal + multiply
  projT = Wo.T @ attnT; y = (projT + bo) + imgT  (one DVE scalar_tensor_tensor)
  layernorm over C: sum / sum-sq via [128,128] all-ones matmuls -> stats
  replicated across all partitions (no broadcast step, no 1-partition ops);
  y^2 on the GPSIMD engine; rstd via ACT Rsqrt.
"""

import numpy as np
from contextlib import ExitStack

import concourse.bass as bass
import concourse.mybir as mybir
import concourse.tile as tile
from concourse import bacc
from concourse.bass_utils import run_bass_kernel_spmd
from concourse.masks import make_identity

# Problem constants (hardcoded per spec)
B, C_IMG, H, W = 16, 512, 32, 32
C_AUD, K_LEN, N_HEADS = 512, 256, 8
HD = C_AUD // N_HEADS           # 64
HW = H * W                      # 1024
EPS = 1e-5
SCALE = float(HD) ** -0.5       # 0.125
N_CORES = 8
BPC = B // N_CORES              # 2 batches per core

F32 = mybir.dt.float32
BF16 = mybir.dt.bfloat16
P = 128
NCI = C_IMG // P                # 4 c_img chunks
NCA = C_AUD // P                # 4 c_aud chunks
NS = K_LEN // P                 # 2 seq chunks
NT = 512                        # matmul free-dim tile (one PSUM bank fp32)
NQ = HW // NT                   # 2 q tiles

# Matmul dtype mode: "f32" (exact, 4 cyc/row), "f32r" (fast, reduced
# precision), or "bf16" (fast, half the DMA/SBUF footprint)
MM_MODE = "f32r"

Ident = mybir.ActivationFunctionType.Identity
Copy = mybir.ActivationFunctionType.Copy
Exp = mybir.ActivationFunctionType.Exp
Rsqrt = mybir.ActivationFunctionType.Rsqrt
ADD = mybir.AluOpType.add
SUB = mybir.AluOpType.subtract
MUL = mybir.AluOpType.mult


def _body(ctx: ExitStack, tc: tile.TileContext, mm_dt, repeat=1):
    nc = tc.nc

    MM = mm_dt                     # dtype for tiles feeding matmuls
    def ff(ap):
        # view an MM-typed AP as plain f32 for non-matmul consumers
        return ap.bitcast(F32) if MM == mybir.dt.float32r else ap

    img_d = nc.dram_tensor("img", [BPC, C_IMG, HW], MM, kind="ExternalInput").ap()
    aud_d = nc.dram_tensor("aud", [BPC, K_LEN, C_AUD], F32, kind="ExternalInput").ap()
    wq_d = nc.dram_tensor("wq", [C_IMG, C_AUD], MM, kind="ExternalInput").ap()
    wk_d = nc.dram_tensor("wk", [C_AUD, C_AUD], MM, kind="ExternalInput").ap()
    wv_d = nc.dram_tensor("wv", [C_AUD, C_AUD], MM, kind="ExternalInput").ap()
    wo_d = nc.dram_tensor("wo", [C_AUD, C_IMG], MM, kind="ExternalInput").ap()
    bq_d = nc.dram_tensor("bq", [C_AUD], F32, kind="ExternalInput").ap()
    bk_d = nc.dram_tensor("bk", [C_AUD], F32, kind="ExternalInput").ap()
    bv_d = nc.dram_tensor("bv", [C_AUD], MM, kind="ExternalInput").ap()
    bo_d = nc.dram_tensor("bo", [C_IMG], F32, kind="ExternalInput").ap()
    gam_d = nc.dram_tensor("gamma", [C_IMG], F32, kind="ExternalInput").ap()
    bet_d = nc.dram_tensor("beta", [C_IMG], F32, kind="ExternalInput").ap()
    out_d = nc.dram_tensor("out", [BPC, C_IMG, HW], F32, kind="ExternalOutput").ap()

    cpool = ctx.enter_context(tc.tile_pool(name="consts", bufs=1))
    wpool = ctx.enter_context(tc.tile_pool(name="weights", bufs=1))
    img_pool = ctx.enter_context(tc.tile_pool(name="img", bufs=2))
    aud_pool = ctx.enter_context(tc.tile_pool(name="aud", bufs=2))
    big_pool = ctx.enter_context(tc.tile_pool(name="big", bufs=1))       # qT, attnT, y
    kv_pool = ctx.enter_context(tc.tile_pool(name="kv", bufs=1))         # kT, audT, v
    expS_pool = ctx.enter_context(tc.tile_pool(name="expS", bufs=6))
    rb_pool = ctx.enter_context(tc.tile_pool(name="rbcast", bufs=3))
    stat_pool = ctx.enter_context(tc.tile_pool(name="stats", bufs=2))    # nmu/e2/rstd
    chunk_pool = ctx.enter_context(tc.tile_pool(name="chunk", bufs=4))   # ysq/tmp/out
    mm_ps = ctx.enter_context(tc.tile_pool(name="mm_ps", bufs=6, space="PSUM"))
    fr_ps = ctx.enter_context(tc.tile_pool(name="fr_ps", bufs=2, space="PSUM"))

    # ---- constants / weights (loaded once) ----
    # memset doesn't codegen for f32r; stage in f32 then ACT-copy (which rounds)
    ones_f32 = cpool.tile([P, P + 1], F32, tag="ones_f32")
    nc.vector.memset(ones_f32[:], 1.0)
    allones = cpool.tile([P, P], MM, tag="allones")
    nc.scalar.activation(allones[:], ones_f32[:, 0:P], Copy)
    ones_row = cpool.tile([1, P], MM)
    nc.scalar.activation(ones_row[:], ones_f32[0:1, 1:P + 1], Copy)
    ones_bf64 = cpool.tile([P, HD], BF16, tag="ones_bf")
    nc.vector.memset(ones_bf64[:], 1.0)
    ident = cpool.tile([P, P], F32)
    make_identity(nc, ident[:])
    eps_col = cpool.tile([P, 1], F32, tag="eps")
    nc.vector.memset(eps_col[:], EPS)

    wq_sb = wpool.tile([P, NCI, C_AUD], MM, tag="wq")
    wk_sb = wpool.tile([P, NCA, C_AUD], MM, tag="wk")
    wv_sb = wpool.tile([P, NCA, C_AUD], MM, tag="wv")
    wo_sb = wpool.tile([P, NCA, C_IMG], MM, tag="wo")
    bq_col = cpool.tile([P, NCA], F32, tag="bq")
    bk_col = cpool.tile([P, NCA], F32, tag="bk")
    bo_col = cpool.tile([P, NCI], F32, tag="bo")
    gam_col = cpool.tile([P, NCI], F32, tag="gam")
    bet_col = cpool.tile([P, NCI], F32, tag="bet")
    bv_row = cpool.tile([1, C_AUD], MM, tag="bv")

    for rep in range(repeat):
        # input tiles for both batches; DMA emission order matters: feed the
        # first dependency chains (audio -> transposes -> kT/v needs wk/wv,
        # qT needs wq+img) in that order.
        aud_tiles, img_tiles = [], []
        for b in range(BPC):
            aud_tiles.append(aud_pool.tile([P, NS, C_AUD], F32, tag="aud",
                                           name=f"aud_sb{b}"))
            img_tiles.append(img_pool.tile([P, NCI, HW], MM, tag="img",
                                           name=f"img_sb{b}"))
        for st in range(NS):
            nc.sync.dma_start(out=aud_tiles[0][:, st, :], in_=aud_d[0, st * P:(st + 1) * P, :])
        for m in range(NCA):
            nc.sync.dma_start(out=bq_col[:, m:m + 1], in_=bq_d[m * P:(m + 1) * P])
            nc.sync.dma_start(out=bk_col[:, m:m + 1], in_=bk_d[m * P:(m + 1) * P])
        # fold the attention scale into q's bias: q_scaled = psum*SCALE + bq*SCALE
        nc.vector.tensor_scalar_mul(bq_col[:], bq_col[:], SCALE)
        for ci in range(NCA):
            nc.sync.dma_start(out=wk_sb[:, ci, :], in_=wk_d[ci * P:(ci + 1) * P, :])
        for ci in range(NCI):
            nc.sync.dma_start(out=wq_sb[:, ci, :], in_=wq_d[ci * P:(ci + 1) * P, :])
        for ci in range(NCI):
            nc.sync.dma_start(out=img_tiles[0][:, ci, :], in_=img_d[0, ci * P:(ci + 1) * P, :])
        for ci in range(NCA):
            nc.sync.dma_start(out=wv_sb[:, ci, :], in_=wv_d[ci * P:(ci + 1) * P, :])
        nc.sync.dma_start(out=bv_row[:], in_=bv_d[:])
        for ci in range(NCA):
            nc.sync.dma_start(out=wo_sb[:, ci, :], in_=wo_d[ci * P:(ci + 1) * P, :])
        for m in range(NCI):
            nc.sync.dma_start(out=bo_col[:, m:m + 1], in_=bo_d[m * P:(m + 1) * P])
            nc.sync.dma_start(out=gam_col[:, m:m + 1], in_=gam_d[m * P:(m + 1) * P])
            nc.sync.dma_start(out=bet_col[:, m:m + 1], in_=bet_d[m * P:(m + 1) * P])
        for b in range(1, BPC):
            for st in range(NS):
                nc.sync.dma_start(out=aud_tiles[b][:, st, :], in_=aud_d[b, st * P:(st + 1) * P, :])
            for ci in range(NCI):
                nc.sync.dma_start(out=img_tiles[b][:, ci, :], in_=img_d[b, ci * P:(ci + 1) * P, :])

        for b in range(BPC):
            img_sb = img_tiles[b]
            aud_sb = aud_tiles[b]

            # ---- audT: transpose audio [s, c] -> [c, s] via PE ----
            audT_sb = kv_pool.tile([P, NCA, K_LEN], MM, tag="audT")
            for ci in range(NCA):
                tp = fr_ps.tile([P, K_LEN], F32, tag="fr")
                for st in range(NS):
                    nc.tensor.transpose(
                        tp[:, st * P:(st + 1) * P],
                        aud_sb[:, st, ci * P:(ci + 1) * P],
                        ident[:],
                    )
                nc.vector.tensor_copy(audT_sb[:, ci, :], tp[:])

            # ---- qT = Wq.T @ imgT, scaled + bias ----
            qT_sb = big_pool.tile([P, NCA, HW], MM, tag="qT", bufs=2)
            for m in range(NCA):
                for n in range(NQ):
                    ps = fr_ps.tile([P, NT], F32, tag="fr")
                    for ci in range(NCI):
                        nc.tensor.matmul(
                            ps[:],
                            wq_sb[:, ci, m * P:(m + 1) * P],
                            img_sb[:, ci, n * NT:(n + 1) * NT],
                            start=(ci == 0), stop=(ci == NCI - 1),
                        )
                    nc.scalar.activation(qT_sb[:, m, n * NT:(n + 1) * NT], ps[:], Ident,
                                         bias=bq_col[:, m:m + 1], scale=SCALE)

            # ---- kT = Wk.T @ audT + bk ----
            kT_sb = kv_pool.tile([P, NCA, K_LEN], MM, tag="kT", bufs=2)
            for m in range(NCA):
                ps = fr_ps.tile([P, K_LEN], F32, tag="fr")
                for ci in range(NCA):
                    nc.tensor.matmul(
                        ps[:],
                        wk_sb[:, ci, m * P:(m + 1) * P],
                        audT_sb[:, ci, :],
                        start=(ci == 0), stop=(ci == NCA - 1),
                    )
                nc.scalar.activation(kT_sb[:, m, :], ps[:], Ident, bias=bk_col[:, m:m + 1])

            # ---- v = audT.T @ Wv + bv (seq-major) ----
            v_sb = kv_pool.tile([P, NS, C_AUD], BF16, tag="v", bufs=2)
            for st in range(NS):
                ps = fr_ps.tile([P, C_AUD], F32, tag="fr")
                for ci in range(NCA):
                    nc.tensor.matmul(
                        ps[:],
                        audT_sb[:, ci, st * P:(st + 1) * P],
                        wv_sb[:, ci, :],
                        start=(ci == 0), stop=False,
                    )
                nc.tensor.matmul(ps[:], ones_row[:], bv_row[:],
                                 start=False, stop=True)
                nc.vector.tensor_copy(v_sb[:, st, :], ps[:])

            # ---- attention, head pairs (2t, 2t+1) ----
            attnT_sb = big_pool.tile([P, NCA, HW], MM, tag="attnT")
            for t in range(N_HEADS // 2):
                expS = []  # [hh][kt] sbuf tiles [128, HW]
                for hh in range(2):
                    h = 2 * t + hh
                    ht, hr = h // 2, (h % 2) * HD
                    eh = []
                    for kt in range(NS):
                        et = expS_pool.tile([P, HW], BF16, tag="expS")
                        for n in range(NQ):
                            sps = mm_ps.tile([P, NT], F32, tag="ps")
                            nc.tensor.matmul(
                                sps[:],
                                kT_sb[hr:hr + HD, ht, kt * P:(kt + 1) * P],
                                qT_sb[hr:hr + HD, ht, n * NT:(n + 1) * NT],
                                start=True, stop=True,
                            )
                            nc.scalar.activation(et[:, n * NT:(n + 1) * NT], sps[:], Exp)
                        eh.append(et)
                    expS.append(eh)

                # attn (col-tiled pairs) + replicated sumexp rows (matmul with an
                # all-ones stationary does the reduction AND the partition
                # broadcast in one shot -> everything stays lane-aligned)
                apss, sebcs = [], []
                for n in range(NQ):
                    aps = mm_ps.tile([P, NT], F32, tag="ps")
                    sebc = mm_ps.tile([P, NT], F32, tag="ps")
                    for hh in range(2):
                        h = 2 * t + hh
                        r0 = hh * HD
                        for kt in range(NS):
                            nc.tensor.matmul(
                                aps[r0:r0 + HD, :],
                                v_sb[:, kt, h * HD:(h + 1) * HD],
                                expS[hh][kt][:, n * NT:(n + 1) * NT],
                                start=(kt == 0), stop=(kt == NS - 1),
                                tile_position=(0, r0),
                            )
                            nc.tensor.matmul(
                                sebc[r0:r0 + HD, :],
                                ones_bf64[:],
                                expS[hh][kt][:, n * NT:(n + 1) * NT],
                                start=(kt == 0), stop=(kt == NS - 1),
                                tile_position=(0, r0),
                            )
                    apss.append(aps)
                    sebcs.append(sebc)

                # normalize: attnT_chunk = attn_pair_psum * recip(sumexp_bcast)
                rb = rb_pool.tile([P, HW], F32, tag="rb")
                for n in range(NQ):
                    nc.vector.reciprocal(rb[:, n * NT:(n + 1) * NT], sebcs[n][:])
                for n in range(NQ):
                    nc.vector.tensor_tensor(attnT_sb[:, t, n * NT:(n + 1) * NT],
                                            apss[n][:], rb[:, n * NT:(n + 1) * NT], MUL)

            # ---- projT = Wo.T @ attnT ; y = (projT + bo) + imgT via one STT ----
            # reuse qT's slot: qT is fully consumed by the score matmuls above
            y_sb = big_pool.tile([P, NCI, HW], MM, tag="qT", bufs=2)
            for m in range(NCI):
                for n in range(NQ):
                    ps = mm_ps.tile([P, NT], F32, tag="ps")
                    for ci in range(NCA):
                        nc.tensor.matmul(
                            ps[:],
                            wo_sb[:, ci, m * P:(m + 1) * P],
                            attnT_sb[:, ci, n * NT:(n + 1) * NT],
                            start=(ci == 0), stop=(ci == NCA - 1),
                        )
                    nc.vector.scalar_tensor_tensor(
                        ff(y_sb[:, m, n * NT:(n + 1) * NT]),
                        ps[:], bo_col[:, m:m + 1],
                        ff(img_sb[:, m, n * NT:(n + 1) * NT]),
                        ADD, ADD,
                    )

            # ---- layernorm stats over C via all-ones matmuls: the [128,128]
            # ones stationary reduces over partitions AND replicates the sums
            # to every partition (matmul cost is free-dim only, so M=128 costs
            # the same as M=1) -> no broadcast step, no 1-partition row ops.
            sum_ps_n = [mm_ps.tile([P, NT], F32, tag="ps", name=f"sum_ps{b}_{i}") for i in range(NQ)]
            for ci in range(NCI):
                for n in range(NQ):
                    nc.tensor.matmul(
                        sum_ps_n[n][:], allones[:], y_sb[:, ci, n * NT:(n + 1) * NT],
                        start=(ci == 0), stop=(ci == NCI - 1),
                    )
            sq_ps_n = [mm_ps.tile([P, NT], F32, tag="ps", name=f"sq_ps{b}_{i}") for i in range(NQ)]
            for ci in range(NCI):
                # y^2 on the (otherwise idle) GPSIMD engine
                ysq = chunk_pool.tile([P, HW], F32, tag="chunk")
                nc.gpsimd.tensor_mul(ysq[:], ff(y_sb[:, ci, :]), ff(y_sb[:, ci, :]))
                ysq_mm = ysq[:].bitcast(MM) if MM == mybir.dt.float32r else ysq[:]
                for n in range(NQ):
                    nc.tensor.matmul(
                        sq_ps_n[n][:], allones[:], ysq_mm[:, n * NT:(n + 1) * NT],
                        start=(ci == 0), stop=(ci == NCI - 1),
                    )

            # stats, replicated across all 128 partitions: nmu = -mean,
            # e2pe = E[y^2] + eps (eps folded into the evacuation bias)
            nmu_bc = stat_pool.tile([P, HW], F32, tag="st")
            e2pe = stat_pool.tile([P, HW], F32, tag="st")
            for n in range(NQ):
                nc.scalar.activation(nmu_bc[:, n * NT:(n + 1) * NT], sum_ps_n[n][:],
                                     Ident, scale=-1.0 / C_IMG)
                nc.scalar.activation(e2pe[:, n * NT:(n + 1) * NT], sq_ps_n[n][:],
                                     Ident, scale=1.0 / C_IMG, bias=eps_col[:])
            mu2 = chunk_pool.tile([P, HW], F32, tag="chunk")
            nc.vector.tensor_tensor(mu2[:], nmu_bc[:], nmu_bc[:], MUL)
            var = chunk_pool.tile([P, HW], F32, tag="chunk")
            nc.vector.tensor_tensor(var[:], e2pe[:], mu2[:], SUB)
            rstd_bc = stat_pool.tile([P, HW], F32, tag="st")
            nc.scalar.activation(rstd_bc[:], var[:], Rsqrt)

            # ---- apply: out = gamma * (y - mu) * rstd + beta ----
            for ci in range(NCI):
                t1 = chunk_pool.tile([P, HW], F32, tag="chunk")
                nc.vector.tensor_tensor(t1[:], ff(y_sb[:, ci, :]), nmu_bc[:], ADD)
                t2 = chunk_pool.tile([P, HW], F32, tag="chunk")
                nc.vector.tensor_tensor(t2[:], t1[:], rstd_bc[:], MUL)
                o = chunk_pool.tile([P, HW], F32, tag="chunk")
                nc.vector.tensor_scalar(out=o[:], in0=t2[:],
                                        scalar1=gam_col[:, ci:ci + 1],
                                        scalar2=bet_col[:, ci:ci + 1],
                                        op0=MUL, op1=ADD)
                nc.sync.dma_start(out=out_d[b, ci * P:(ci + 1) * P, :], in_=o[:])


def build(mm_mode=MM_MODE, repeat=1):
    mm_dt = {"f32": F32, "f32r": mybir.dt.float32r, "bf16": BF16}[mm_mode]
    nc = bacc.Bacc("TRN2", target_bir_lowering=False, debug=False)
    with tile.TileContext(nc) as tc, ExitStack() as ctx:
        _body(ctx, tc, mm_dt, repeat=repeat)
    nc.compile()
    return nc


_NC_CACHE = {}


def _get_nc(mm_mode=MM_MODE):
    if mm_mode not in _NC_CACHE:
        _NC_CACHE[mm_mode] = build(mm_mode)
    return _NC_CACHE[mm_mode]


def _in_maps(inputs, mm_mode=MM_MODE):
    img = np.ascontiguousarray(np.asarray(inputs["img_feat"], np.float32)
                               .reshape(B, C_IMG, HW))
    aud = np.ascontiguousarray(np.asarray(inputs["audio_feat"], np.float32))
    shared = {
        "wq": np.asarray(inputs["Wq"], np.float32),
        "wk": np.asarray(inputs["Wk"], np.float32),
        "wv": np.asarray(inputs["Wv"], np.float32),
        "wo": np.asarray(inputs["Wo"], np.float32),
        "bq": np.asarray(inputs["bq"], np.float32),
        "bk": np.asarray(inputs["bk"], np.float32),
        "bv": np.asarray(inputs["bv"], np.float32),
        "bo": np.asarray(inputs["bo"], np.float32),
        "gamma": np.asarray(inputs["gamma"], np.float32),
        "beta": np.asarray(inputs["beta"], np.float32),
    }
    if mm_mode == "bf16":
        import ml_dtypes
        bf = ml_dtypes.bfloat16
        img = img.astype(bf)
        for k in ("wq", "wk", "wv", "wo", "bv"):
            shared[k] = shared[k].astype(bf)
    maps = []
    for c in range(N_CORES):
        sl = slice(c * BPC, (c + 1) * BPC)
        maps.append({"img": img[sl], "aud": aud[sl], **shared})
    return maps


def kernel(**inputs) -> np.ndarray:
    nc = _get_nc()
    res = run_bass_kernel_spmd(nc, _in_maps(inputs, MM_MODE), list(range(N_CORES)))
    outs = [res.results[c]["out"] for c in range(N_CORES)]
    return np.concatenate(outs, axis=0).reshape(B, C_IMG, H, W)


def kernel_profiled(inputs, mm_mode=MM_MODE, **kw):
    """Returns (output, BassKernelResults). NTFF tracing is unavailable in this
    container (axon.trn not shipped), so exec_time_ns is None; use test.py's
    repeated-call timing instead."""
    nc = _get_nc(mm_mode)
    res = run_bass_kernel_spmd(nc, _in_maps(inputs, mm_mode), list(range(N_CORES)), **kw)
    outs = [res.results[c]["out"] for c in range(N_CORES)]
    return np.concatenate(outs, axis=0).reshape(B, C_IMG, H, W), res


# revision 13
# speedup vs baseline: 669.3324x; 669.3324x over previous
"""Fused cross-attention audio fuser (dense transformer block) on TRN2.

Strategy: pure batch data-parallelism across the 8 NeuronCores (B=16 -> 2
batches per core, no collectives). Per batch everything is kept
channel-major ([C, tokens]) so the only transposes needed are 8 PE block
transposes of the audio features:

  qT = Wq.T @ imgT (+bq, *scale)      [C_AUD, HW]     (imgT is native layout)
  kT = Wk.T @ audT (+bk)              [C_AUD, K_LEN]
  v  = audT.T @ Wv (+bv via K=1 mm)   [K_LEN, C_AUD]  (seq-major)
  S_hT = kT_h.T @ qT_h                [K_LEN, HW] per head (K=64, row-packed pairs)
  expS = exp(S_hT)                    (no max subtraction; scores are provably small)
  sumexp_hT = ones.T @ expS           (matmul reduction+partition broadcast in one)
  attnT_h = v_h.T @ expS              (col-tiled head pairs -> [128, HW] chunks)
  attnT normalized by DVE reciprocal + multiply
  projT = Wo.T @ attnT; y = (projT + bo) + imgT  (one DVE scalar_tensor_tensor)
  layernorm over C: sum / sum-sq via [128,128] all-ones matmuls -> stats
  replicated across all partitions (no broadcast step, no 1-partition ops);
  y^2 on the GPSIMD engine; rstd via ACT Rsqrt.
"""

import numpy as np
from contextlib import ExitStack

import concourse.bass as bass
import concourse.mybir as mybir
import concourse.tile as tile
from concourse import bacc
from concourse.bass_utils import run_bass_kernel_spmd
from concourse.masks import make_identity

# Problem constants (hardcoded per spec)
B, C_IMG, H, W = 16, 512, 32, 32
C_AUD, K_LEN, N_HEADS = 512, 256, 8
HD = C_AUD // N_HEADS           # 64
HW = H * W                      # 1024
EPS = 1e-5
SCALE = float(HD) ** -0.5       # 0.125
N_CORES = 8
BPC = B // N_CORES              # 2 batches per core

F32 = mybir.dt.float32
BF16 = mybir.dt.bfloat16
P = 128
NCI = C_IMG // P                # 4 c_img chunks
NCA = C_AUD // P                # 4 c_aud chunks
NS = K_LEN // P                 # 2 seq chunks
NT = 512                        # matmul free-dim tile (one PSUM bank fp32)
NQ = HW // NT                   # 2 q tiles

# Matmul dtype mode: "f32" (exact, 4 cyc/row), "f32r" (fast, reduced
# precision), or "bf16" (fast, half the DMA/SBUF footprint)
MM_MODE = "f32r"

Ident = mybir.ActivationFunctionType.Identity
Copy = mybir.ActivationFunctionType.Copy
Exp = mybir.ActivationFunctionType.Exp
Sqrt = mybir.ActivationFunctionType.Sqrt
ADD = mybir.AluOpType.add
SUB = mybir.AluOpType.subtract
MUL = mybir.AluOpType.mult


def _body(ctx: ExitStack, tc: tile.TileContext, mm_dt, repeat=1):
    nc = tc.nc

    MM = mm_dt                     # dtype for tiles feeding matmuls
    def ff(ap):
        # view an MM-typed AP as plain f32 for non-matmul consumers
        return ap.bitcast(F32) if MM == mybir.dt.float32r else ap

    img_d = nc.dram_tensor("img", [BPC, C_IMG, HW], MM, kind="ExternalInput").ap()
    aud_d = nc.dram_tensor("aud", [BPC, K_LEN, C_AUD], F32, kind="ExternalInput").ap()
    wq_d = nc.dram_tensor("wq", [C_IMG, C_AUD], MM, kind="ExternalInput").ap()
    wk_d = nc.dram_tensor("wk", [C_AUD, C_AUD], MM, kind="ExternalInput").ap()
    wv_d = nc.dram_tensor("wv", [C_AUD, C_AUD], MM, kind="ExternalInput").ap()
    wo_d = nc.dram_tensor("wo", [C_AUD, C_IMG], MM, kind="ExternalInput").ap()
    bq_d = nc.dram_tensor("bq", [C_AUD], F32, kind="ExternalInput").ap()
    bk_d = nc.dram_tensor("bk", [C_AUD], F32, kind="ExternalInput").ap()
    bv_d = nc.dram_tensor("bv", [C_AUD], MM, kind="ExternalInput").ap()
    bo_d = nc.dram_tensor("bo", [C_IMG], F32, kind="ExternalInput").ap()
    gam_d = nc.dram_tensor("gamma", [C_IMG], F32, kind="ExternalInput").ap()
    bet_d = nc.dram_tensor("beta", [C_IMG], F32, kind="ExternalInput").ap()
    out_d = nc.dram_tensor("out", [BPC, C_IMG, HW], F32, kind="ExternalOutput").ap()

    cpool = ctx.enter_context(tc.tile_pool(name="consts", bufs=1))
    wpool = ctx.enter_context(tc.tile_pool(name="weights", bufs=1))
    img_pool = ctx.enter_context(tc.tile_pool(name="img", bufs=2))
    aud_pool = ctx.enter_context(tc.tile_pool(name="aud", bufs=2))
    big_pool = ctx.enter_context(tc.tile_pool(name="big", bufs=1))       # qT, attnT, y
    kv_pool = ctx.enter_context(tc.tile_pool(name="kv", bufs=1))         # kT, audT, v
    expS_pool = ctx.enter_context(tc.tile_pool(name="expS", bufs=6))
    rb_pool = ctx.enter_context(tc.tile_pool(name="rbcast", bufs=3))
    stat_pool = ctx.enter_context(tc.tile_pool(name="stats", bufs=2))    # nmu/e2/rstd
    chunk_pool = ctx.enter_context(tc.tile_pool(name="chunk", bufs=4))   # ysq/tmp/out
    mm_ps = ctx.enter_context(tc.tile_pool(name="mm_ps", bufs=6, space="PSUM"))
    fr_ps = ctx.enter_context(tc.tile_pool(name="fr_ps", bufs=2, space="PSUM"))

    # ---- constants / weights (loaded once) ----
    # memset doesn't codegen for f32r; stage in f32 then ACT-copy (which rounds)
    ones_f32 = cpool.tile([P, P + 1], F32, tag="ones_f32")
    nc.vector.memset(ones_f32[:], 1.0)
    allones = cpool.tile([P, P], MM, tag="allones")
    nc.scalar.activation(allones[:], ones_f32[:, 0:P], Copy)
    ones_row = cpool.tile([1, P], MM)
    nc.scalar.activation(ones_row[:], ones_f32[0:1, 1:P + 1], Copy)
    ones_bf64 = cpool.tile([P, HD], BF16, tag="ones_bf")
    nc.vector.memset(ones_bf64[:], 1.0)
    ident = cpool.tile([P, P], F32)
    make_identity(nc, ident[:])
    eps_col = cpool.tile([P, 1], F32, tag="eps")
    nc.vector.memset(eps_col[:], EPS)

    wq_sb = wpool.tile([P, NCI, C_AUD], MM, tag="wq")
    wk_sb = wpool.tile([P, NCA, C_AUD], MM, tag="wk")
    wv_sb = wpool.tile([P, NCA, C_AUD], MM, tag="wv")
    wo_sb = wpool.tile([P, NCA, C_IMG], MM, tag="wo")
    bq_col = cpool.tile([P, NCA], F32, tag="bq")
    bk_col = cpool.tile([P, NCA], F32, tag="bk")
    bo_col = cpool.tile([P, NCI], F32, tag="bo")
    gam_col = cpool.tile([P, NCI], F32, tag="gam")
    bet_col = cpool.tile([P, NCI], F32, tag="bet")
    bv_row = cpool.tile([1, C_AUD], MM, tag="bv")

    for rep in range(repeat):
        # input tiles for both batches; DMA emission order matters: feed the
        # first dependency chains (audio -> transposes -> kT/v needs wk/wv,
        # qT needs wq+img) in that order.
        aud_tiles, img_tiles = [], []
        for b in range(BPC):
            aud_tiles.append(aud_pool.tile([P, NS, C_AUD], F32, tag="aud",
                                           name=f"aud_sb{b}"))
            img_tiles.append(img_pool.tile([P, NCI, HW], MM, tag="img",
                                           name=f"img_sb{b}"))
        for st in range(NS):
            nc.sync.dma_start(out=aud_tiles[0][:, st, :], in_=aud_d[0, st * P:(st + 1) * P, :])
        for ci in range(NCA):
            nc.sync.dma_start(out=wk_sb[:, ci, :], in_=wk_d[ci * P:(ci + 1) * P, :])
        for m in range(NCA):
            nc.sync.dma_start(out=bq_col[:, m:m + 1], in_=bq_d[m * P:(m + 1) * P])
            nc.sync.dma_start(out=bk_col[:, m:m + 1], in_=bk_d[m * P:(m + 1) * P])
        # fold the attention scale into q's bias: q_scaled = psum*SCALE + bq*SCALE
        nc.vector.tensor_scalar_mul(bq_col[:], bq_col[:], SCALE)
        for ci in range(NCA):
            nc.sync.dma_start(out=wv_sb[:, ci, :], in_=wv_d[ci * P:(ci + 1) * P, :])
        nc.sync.dma_start(out=bv_row[:], in_=bv_d[:])
        for ci in range(NCI):
            nc.sync.dma_start(out=wq_sb[:, ci, :], in_=wq_d[ci * P:(ci + 1) * P, :])
        for ci in range(NCI):
            nc.sync.dma_start(out=img_tiles[0][:, ci, :], in_=img_d[0, ci * P:(ci + 1) * P, :])
        for ci in range(NCA):
            nc.sync.dma_start(out=wo_sb[:, ci, :], in_=wo_d[ci * P:(ci + 1) * P, :])
        for m in range(NCI):
            nc.sync.dma_start(out=bo_col[:, m:m + 1], in_=bo_d[m * P:(m + 1) * P])
            nc.sync.dma_start(out=gam_col[:, m:m + 1], in_=gam_d[m * P:(m + 1) * P])
            nc.sync.dma_start(out=bet_col[:, m:m + 1], in_=bet_d[m * P:(m + 1) * P])
        for b in range(1, BPC):
            for st in range(NS):
                nc.sync.dma_start(out=aud_tiles[b][:, st, :], in_=aud_d[b, st * P:(st + 1) * P, :])
            for ci in range(NCI):
                nc.sync.dma_start(out=img_tiles[b][:, ci, :], in_=img_d[b, ci * P:(ci + 1) * P, :])

        for b in range(BPC):
            img_sb = img_tiles[b]
            aud_sb = aud_tiles[b]

            # ---- audT: transpose audio [s, c] -> [c, s] via PE ----
            audT_sb = kv_pool.tile([P, NCA, K_LEN], MM, tag="audT")
            for ci in range(NCA):
                tp = fr_ps.tile([P, K_LEN], F32, tag="fr")
                for st in range(NS):
                    nc.tensor.transpose(
                        tp[:, st * P:(st + 1) * P],
                        aud_sb[:, st, ci * P:(ci + 1) * P],
                        ident[:],
                    )
                nc.vector.tensor_copy(audT_sb[:, ci, :], tp[:])

            # ---- kT = Wk.T @ audT + bk ----
            kT_sb = kv_pool.tile([P, NCA, K_LEN], MM, tag="kT", bufs=2)
            for m in range(NCA):
                ps = fr_ps.tile([P, K_LEN], F32, tag="fr")
                for ci in range(NCA):
                    nc.tensor.matmul(
                        ps[:],
                        wk_sb[:, ci, m * P:(m + 1) * P],
                        audT_sb[:, ci, :],
                        start=(ci == 0), stop=(ci == NCA - 1),
                    )
                nc.scalar.activation(kT_sb[:, m, :], ps[:], Ident, bias=bk_col[:, m:m + 1])

            # ---- v = audT.T @ Wv + bv (seq-major) ----
            v_sb = kv_pool.tile([P, NS, C_AUD], BF16, tag="v", bufs=2)
            for st in range(NS):
                ps = fr_ps.tile([P, C_AUD], F32, tag="fr")
                for ci in range(NCA):
                    nc.tensor.matmul(
                        ps[:],
                        audT_sb[:, ci, st * P:(st + 1) * P],
                        wv_sb[:, ci, :],
                        start=(ci == 0), stop=False,
                    )
                nc.tensor.matmul(ps[:], ones_row[:], bv_row[:],
                                 start=False, stop=True)
                nc.vector.tensor_copy(v_sb[:, st, :], ps[:])

            # ---- qT = Wq.T @ imgT, scaled + bias ----
            qT_sb = big_pool.tile([P, NCA, HW], MM, tag="qT", bufs=2)
            for m in range(NCA):
                for n in range(NQ):
                    ps = fr_ps.tile([P, NT], F32, tag="fr")
                    for ci in range(NCI):
                        nc.tensor.matmul(
                            ps[:],
                            wq_sb[:, ci, m * P:(m + 1) * P],
                            img_sb[:, ci, n * NT:(n + 1) * NT],
                            start=(ci == 0), stop=(ci == NCI - 1),
                        )
                    nc.scalar.activation(qT_sb[:, m, n * NT:(n + 1) * NT], ps[:], Ident,
                                         bias=bq_col[:, m:m + 1], scale=SCALE)

            # ---- attention, head pairs (2t, 2t+1) ----
            attnT_sb = big_pool.tile([P, NCA, HW], MM, tag="attnT")
            for t in range(N_HEADS // 2):
                expS = []  # [hh][kt] sbuf tiles [128, HW]
                for hh in range(2):
                    h = 2 * t + hh
                    ht, hr = h // 2, (h % 2) * HD
                    eh = []
                    for kt in range(NS):
                        et = expS_pool.tile([P, HW], BF16, tag="expS")
                        for n in range(NQ):
                            sps = mm_ps.tile([P, NT], F32, tag="ps")
                            nc.tensor.matmul(
                                sps[:],
                                kT_sb[hr:hr + HD, ht, kt * P:(kt + 1) * P],
                                qT_sb[hr:hr + HD, ht, n * NT:(n + 1) * NT],
                                start=True, stop=True,
                            )
                            nc.scalar.activation(et[:, n * NT:(n + 1) * NT], sps[:], Exp)
                        eh.append(et)
                    expS.append(eh)

                # attn (col-tiled pairs) + replicated sumexp rows (matmul with an
                # all-ones stationary does the reduction AND the partition
                # broadcast in one shot -> everything stays lane-aligned)
                apss, sebcs = [], []
                for n in range(NQ):
                    aps = mm_ps.tile([P, NT], F32, tag="ps")
                    sebc = mm_ps.tile([P, NT], F32, tag="ps")
                    for hh in range(2):
                        h = 2 * t + hh
                        r0 = hh * HD
                        for kt in range(NS):
                            nc.tensor.matmul(
                                aps[r0:r0 + HD, :],
                                v_sb[:, kt, h * HD:(h + 1) * HD],
                                expS[hh][kt][:, n * NT:(n + 1) * NT],
                                start=(kt == 0), stop=(kt == NS - 1),
                                tile_position=(0, r0),
                            )
                            nc.tensor.matmul(
                                sebc[r0:r0 + HD, :],
                                ones_bf64[:],
                                expS[hh][kt][:, n * NT:(n + 1) * NT],
                                start=(kt == 0), stop=(kt == NS - 1),
                                tile_position=(0, r0),
                            )
                    apss.append(aps)
                    sebcs.append(sebc)

                # normalize: attnT_chunk = attn_pair_psum * recip(sumexp_bcast)
                rb = rb_pool.tile([P, HW], F32, tag="rb")
                for n in range(NQ):
                    nc.vector.reciprocal(rb[:, n * NT:(n + 1) * NT], sebcs[n][:])
                for n in range(NQ):
                    nc.vector.tensor_tensor(attnT_sb[:, t, n * NT:(n + 1) * NT],
                                            apss[n][:], rb[:, n * NT:(n + 1) * NT], MUL)

            # ---- projT = Wo.T @ attnT ; y = (projT + bo) + imgT via one STT ----
            # n-major: the whole n=0 half of projT/stats/layernorm/apply
            # completes while PE is still on the n=1 half, hiding the
            # layernorm tail behind matmul work.
            # reuse qT's slot: qT is fully consumed by the score matmuls above
            y_sb = big_pool.tile([P, NCI, HW], MM, tag="qT", bufs=2)
            for n in range(NQ):
                sl = slice(n * NT, (n + 1) * NT)
                for m in range(NCI):
                    ps = mm_ps.tile([P, NT], F32, tag="ps")
                    for ci in range(NCA):
                        nc.tensor.matmul(
                            ps[:],
                            wo_sb[:, ci, m * P:(m + 1) * P],
                            attnT_sb[:, ci, sl],
                            start=(ci == 0), stop=(ci == NCA - 1),
                        )
                    nc.vector.scalar_tensor_tensor(
                        y_sb[:, m, sl],
                        ps[:], bo_col[:, m:m + 1],
                        ff(img_sb[:, m, sl]),
                        ADD, ADD,
                    )

                # layernorm stats over C via all-ones matmuls: the [128,128]
                # ones stationary reduces over partitions AND replicates the
                # sums to every partition (matmul cost is free-dim only, so
                # M=128 costs the same as M=1) -> no broadcast step, no
                # 1-partition row ops.
                sum_ps = mm_ps.tile([P, NT], F32, tag="ps", name=f"sum_ps{b}_{n}")
                for ci in range(NCI):
                    nc.tensor.matmul(
                        sum_ps[:], allones[:], y_sb[:, ci, sl],
                        start=(ci == 0), stop=(ci == NCI - 1),
                    )
                sq_ps = mm_ps.tile([P, NT], F32, tag="ps", name=f"sq_ps{b}_{n}")
                for ci in range(NCI):
                    ysq = chunk_pool.tile([P, NT], MM, tag="ysq")
                    nc.vector.tensor_tensor(ysq[:], ff(y_sb[:, ci, sl]),
                                            ff(y_sb[:, ci, sl]), MUL)
                    nc.tensor.matmul(
                        sq_ps[:], allones[:], ysq[:],
                        start=(ci == 0), stop=(ci == NCI - 1),
                    )

                # stats, replicated across all 128 partitions: nmu = -mean,
                # e2pe = E[y^2] + eps (eps folded into the evacuation bias)
                nmu_bc = stat_pool.tile([P, NT], F32, tag="st")
                nc.scalar.activation(nmu_bc[:], sum_ps[:], Ident, scale=-1.0 / C_IMG)
                e2pe = stat_pool.tile([P, NT], F32, tag="st")
                nc.scalar.activation(e2pe[:], sq_ps[:], Ident,
                                     scale=1.0 / C_IMG, bias=eps_col[:])
                mu2 = chunk_pool.tile([P, NT], F32, tag="chunk")
                nc.vector.tensor_tensor(mu2[:], nmu_bc[:], nmu_bc[:], MUL)
                var = chunk_pool.tile([P, NT], F32, tag="chunk")
                nc.vector.tensor_tensor(var[:], e2pe[:], mu2[:], SUB)
                std = chunk_pool.tile([P, NT], F32, tag="chunk")
                nc.scalar.activation(std[:], var[:], Sqrt)
                rstd_bc = stat_pool.tile([P, NT], F32, tag="st")
                nc.vector.reciprocal(rstd_bc[:], std[:])

                # apply: out = gamma * (y - mu) * rstd + beta
                # spread across engines: t1 on GPSIMD, t2 on DVE, o on ACT
                for ci in range(NCI):
                    t1 = chunk_pool.tile([P, NT], F32, tag="chunk")
                    nc.gpsimd.tensor_add(t1[:], ff(y_sb[:, ci, sl]), nmu_bc[:])
                    t2 = chunk_pool.tile([P, NT], F32, tag="chunk")
                    nc.vector.tensor_tensor(t2[:], t1[:], rstd_bc[:], MUL)
                    o = chunk_pool.tile([P, NT], F32, tag="chunk")
                    nc.scalar.activation(o[:], t2[:], Ident,
                                         scale=gam_col[:, ci:ci + 1],
                                         bias=bet_col[:, ci:ci + 1])
                    nc.sync.dma_start(out=out_d[b, ci * P:(ci + 1) * P, sl], in_=o[:])


def build(mm_mode=MM_MODE, repeat=1):
    mm_dt = {"f32": F32, "f32r": mybir.dt.float32r, "bf16": BF16}[mm_mode]
    nc = bacc.Bacc("TRN2", target_bir_lowering=False, debug=False)
    with tile.TileContext(nc) as tc, ExitStack() as ctx:
        _body(ctx, tc, mm_dt, repeat=repeat)
    nc.compile()
    return nc


_NC_CACHE = {}


def _get_nc(mm_mode=MM_MODE):
    if mm_mode not in _NC_CACHE:
        _NC_CACHE[mm_mode] = build(mm_mode)
    return _NC_CACHE[mm_mode]


def _in_maps(inputs, mm_mode=MM_MODE):
    img = np.ascontiguousarray(np.asarray(inputs["img_feat"], np.float32)
                               .reshape(B, C_IMG, HW))
    aud = np.ascontiguousarray(np.asarray(inputs["audio_feat"], np.float32))
    shared = {
        "wq": np.asarray(inputs["Wq"], np.float32),
        "wk": np.asarray(inputs["Wk"], np.float32),
        "wv": np.asarray(inputs["Wv"], np.float32),
        "wo": np.asarray(inputs["Wo"], np.float32),
        "bq": np.asarray(inputs["bq"], np.float32),
        "bk": np.asarray(inputs["bk"], np.float32),
        "bv": np.asarray(inputs["bv"], np.float32),
        "bo": np.asarray(inputs["bo"], np.float32),
        "gamma": np.asarray(inputs["gamma"], np.float32),
        "beta": np.asarray(inputs["beta"], np.float32),
    }
    if mm_mode == "bf16":
        import ml_dtypes
        bf = ml_dtypes.bfloat16
        img = img.astype(bf)
        for k in ("wq", "wk", "wv", "wo", "bv"):
            shared[k] = shared[k].astype(bf)
    maps = []
    for c in range(N_CORES):
        sl = slice(c * BPC, (c + 1) * BPC)
        maps.append({"img": img[sl], "aud": aud[sl], **shared})
    return maps


def kernel(**inputs) -> np.ndarray:
    nc = _get_nc()
    res = run_bass_kernel_spmd(nc, _in_maps(inputs, MM_MODE), list(range(N_CORES)))
    outs = [res.results[c]["out"] for c in range(N_CORES)]
    return np.concatenate(outs, axis=0).reshape(B, C_IMG, H, W)


def kernel_profiled(inputs, mm_mode=MM_MODE, **kw):
    """Returns (output, BassKernelResults). NTFF tracing is unavailable in this
    container (axon.trn not shipped), so exec_time_ns is None; use test.py's
    repeated-call timing instead."""
    nc = _get_nc(mm_mode)
    res = run_bass_kernel_spmd(nc, _in_maps(inputs, mm_mode), list(range(N_CORES)), **kw)
    outs = [res.results[c]["out"] for c in range(N_CORES)]
    return np.concatenate(outs, axis=0).reshape(B, C_IMG, H, W), res
